# revision 22
# baseline (speedup 1.0000x reference)
"""CARAFE forward as a Bass/Tile kernel for 8 Trainium2 NeuronCores.

Problem (per sample, B=16 total, data-parallel 2 samples/core):
  x [4096, 256] -> down 1x1 conv (256->64) -> enc 3x3 conv (64->36)
  -> pixel_shuffle + softmax over 9 patch weights per upsampled pixel
  -> content-adaptive reassembly of out-conv features -> out [16384, 256]

Key algebraic fusion: the trailing 1x1 conv (out_w) commutes with the
reassembly, so we conv first on the 4096-pixel grid (v = x @ wo + bo; the
softmax weights sum to 1 so the bias passes through), then reassemble v
directly into the final output: 4x fewer conv FLOPs and no y-transpose.

Reassembly is a locally-connected contraction. Per output group of 32
positions x 4 subpixels (M=128) we build a banded weight matrix Mkt
[v-rows, (u,delta)] on-chip with the gpsimd local_scatter primitive
(per-partition scatter), then run 2 accumulated fp16 matmuls against
natural 128-row chunks of v. The scatter's per-partition source values
(kt of each contributing output pixel) are produced by staging softmaxed
kt through DRAM and re-loading with shifted linear access patterns.
"""
import os
import sys
import numpy as np

sys.path.insert(0, "/opt/trn_rl_repo")

import concourse.bass as bass
import concourse.mybir as mybir
import concourse.tile as tile
from concourse.bass_utils import run_bass_kernel_spmd

F32 = mybir.dt.float32
F16 = mybir.dt.float16
I16 = mybir.dt.int16

U, K, C, H, W = 2, 3, 256, 64, 64
HW = H * W                      # 4096
C4 = C // 4                     # 64
NK, NU, NCH = 9, 4, 36
GW = 32                         # output positions per group
PAD = 65                        # kt DRAM row padding (max |shift|)
NCORES = 8
BPC = 2                         # samples per core

_cache = {}
last_result = None


# ---------------------------------------------------------------------------
# host-side constant tables
# ---------------------------------------------------------------------------

def _variant_key(h):
    """(par, border, tA, tB, pB, tlo) for output row h."""
    par = h % 2
    if par == 0:
        tA, tB, pB = h // 2, h // 2 - 1, 1
        if h == 0:
            return par, 'h0', tA, None, None, 0
        tlo = tB
    else:
        tA, tB, pB = (h - 1) // 2, (h + 1) // 2, 0
        if h == H - 1:
            return par, 'h63', tA, None, None, tA
        tlo = tA
    return par, None, tA, tB, pB, tlo


def _build_idx_tables():
    """8 variants of [128, 72] int16 local_scatter index tables.

    Slot s = role*36 + k*4 + u. Data cols span data_sh chunks (tlo, tlo+1).
    even h: role0 -> MM-B (chunk tlo = tA-1), role1 -> MM-A (tA)
    odd  h: role0 -> MM-A (tlo = tA),        role1 -> MM-B (tA+1)
    dst col = (0 for MM-A | 128 for MM-B) + 32*u + delta, delta = wout-w0.
    """
    variants = {}
    for par in (0, 1):
        for w0 in (0, GW):
            for border in (None, 'h0' if par == 0 else 'h63'):
                idx = np.full((128, 72), -1, np.int16)
                for p in range(128):
                    for role in (0, 1):
                        if border is not None:
                            # data span starts at tA; role0 = MM-A, no MM-B
                            if role == 1:
                                continue
                            mm = 'A'
                        elif par == 0:
                            mm = 'B' if role == 0 else 'A'
                        else:
                            mm = 'A' if role == 0 else 'B'
                        if mm == 'A':
                            dh = p // 64 if par == 0 else p // 64 - 1
                            dstbase = 0
                        else:
                            pB = 1 if par == 0 else 0
                            if p // 64 != pB:
                                continue
                            dh = -1 if par == 0 else 1
                            dstbase = 128
                        wr = p % 64
                        for dj in range(3):
                            dw = dj - 1
                            k = (dh + 1) * 3 + dj
                            wout = wr - dw
                            delta = wout - w0
                            if not (0 <= delta < GW and 0 <= wout < W):
                                continue
                            for u in range(NU):
                                # j = 64*ui + 2*delta + uj -> contiguous
                                # upsampled output rows per 64-partition half
                                idx[p, role * 36 + k * 4 + u] = \
                                    dstbase + 64 * (u // 2) + 2 * delta + u % 2
                variants[(par, w0, border)] = idx
    return variants


_IDX_ORDER = [(0, 0, None), (0, GW, None), (1, 0, None), (1, GW, None),
              (0, 0, 'h0'), (0, GW, 'h0'), (1, 0, 'h63'), (1, GW, 'h63')]


def _idx_variant_id(par, w0, border):
    return _IDX_ORDER.index((par, w0, border))


# ---------------------------------------------------------------------------
# device program
# ---------------------------------------------------------------------------

def _build_program():
    nc = bass.Bass()

    x2 = nc.declare_dram_parameter("x2", [BPC, HW, C], F32, isOutput=False)
    wd = nc.declare_dram_parameter("wd", [128, 128], F32, isOutput=False)
    bd = nc.declare_dram_parameter("bd", [C4, 1], F32, isOutput=False)
    weA = nc.declare_dram_parameter("weA", [128, 108], F32, isOutput=False)
    weB = nc.declare_dram_parameter("weB", [C4, 108], F32, isOutput=False)
    be = nc.declare_dram_parameter("be", [NCH, 1], F32, isOutput=False)
    wo = nc.declare_dram_parameter("wo", [128, 512], F32, isOutput=False)
    bo = nc.declare_dram_parameter("bo", [1, C], F32, isOutput=False)
    out = nc.declare_dram_parameter("out", [BPC, 4 * HW, C], F32, True)

    idx_np = _build_idx_tables()
    idx_all = np.concatenate([idx_np[k] for k in _IDX_ORDER], axis=1)  # [128, 576]
    idx_dram = nc.inline_tensor(idx_all, name="idxtab")
    ident_dram = nc.inline_tensor(np.eye(128, dtype=np.float32), name="ident")
    ones_dram = nc.inline_tensor(np.ones((1, 128), np.float32), name="ones1")

    ktd = nc.dram_tensor("ktd", [BPC, HW + 2 * PAD, NCH], F16)

    with tile.TileContext(nc) as tc:
        _emit(tc, nc, x2, wd, bd, weA, weB, be, wo, bo, out,
              idx_dram, ident_dram, ones_dram, ktd)
    # raw-Bass path skips Bacc's extended-inst codegen; without this the
    # NEFF compiler sees empty .instr bytes -> "ISA wrong length"
    from concourse.library_overlay import lower_extended_insts
    lower_extended_insts(nc)
    _split_excess_waits(nc)
    return nc


def _split_excess_waits(nc, cap=1):
    """Each TPB instruction has a single EVENTS wait slot; walrus rejects
    multi-wait instructions ("Too many sync wait commands"). Move excess
    waits onto same-engine NoOps immediately before the instruction —
    semantically identical since the engine blocks at the same PC."""
    nid = [0]
    for f in nc.m.functions:
        for b in f.blocks:
            insts = b.instructions
            i = 0
            while i < len(insts):
                ins = insts[i]
                si = getattr(ins, 'sync_info', None)
                if si is not None and si.on_wait and len(si.on_wait) > cap:
                    waits = list(si.on_wait)
                    for w in waits[:-cap]:
                        nop = mybir.InstNoOp(name=f"nopw-{nid[0]}", ins=[],
                                             outs=[])
                        nid[0] += 1
                        nop.engine = ins.engine
                        nop.sync_info = mybir.SyncInfo(on_wait=[w],
                                                       on_update=[])
                        insts.insert(i, nop)
                        i += 1
                    ins.sync_info = mybir.SyncInfo(
                        on_wait=waits[-cap:],
                        on_update=list(si.on_update or []))
                i += 1


def _emit(tc, nc, x2, wd, bd, weA, weB, be, wo, bo, out,
          idx_dram, ident_dram, ones_dram, ktd):
    from contextlib import ExitStack
    ablate = set(os.environ.get("KABLATE", "").split(","))
    ctx = ExitStack()
    with ctx:
        consts = ctx.enter_context(tc.tile_pool(name="consts", bufs=1))
        xt_pool = ctx.enter_context(tc.tile_pool(name="xt", bufs=2))
        o1_pool = ctx.enter_context(tc.tile_pool(name="o1", bufs=1))
        enc_pool = ctx.enter_context(tc.tile_pool(name="enc", bufs=1))
        kt_pool = ctx.enter_context(tc.tile_pool(name="kt", bufs=1))
        ds_pool = ctx.enter_context(tc.tile_pool(name="ds", bufs=2))
        v_pool = ctx.enter_context(tc.tile_pool(name="v", bufs=2))
        mkt_pool = ctx.enter_context(tc.tile_pool(name="mkt", bufs=6))
        out_pool = ctx.enter_context(tc.tile_pool(name="ob", bufs=6))
        ps_misc = ctx.enter_context(tc.tile_pool(name="psm", bufs=2, space="PSUM"))
        ps_v = ctx.enter_context(tc.tile_pool(name="psv", bufs=2, space="PSUM"))
        ps_o = ctx.enter_context(tc.tile_pool(name="pso", bufs=3, space="PSUM"))

        # ---- constants to SBUF ----
        wd_sb = consts.tile([128, 128], F32)
        nc.sync.dma_start(wd_sb[:], wd[:])
        bd_sb = consts.tile([C4, 1], F32)
        nc.sync.dma_start(bd_sb[:], bd[:])
        weA_sb = consts.tile([128, 108], F32)
        nc.sync.dma_start(weA_sb[:], weA[:])
        weB_sb = consts.tile([C4, 108], F32)
        nc.sync.dma_start(weB_sb[:], weB[:])
        be_sb = consts.tile([NCH, 1], F32)
        nc.sync.dma_start(be_sb[:], be[:])
        wo_sb = consts.tile([128, 512], F32)
        nc.sync.dma_start(wo_sb[:], wo[:])
        bo_sb = consts.tile([1, C], F32)
        nc.sync.dma_start(bo_sb[:], bo[:])
        idx_sb = consts.tile([128, 576], I16)
        nc.sync.dma_start(idx_sb[:], idx_dram[:])
        id_sb = consts.tile([128, 128], F32)
        nc.sync.dma_start(id_sb[:], ident_dram[:])
        ones_sb = consts.tile([1, 128], F32)
        nc.sync.dma_start(ones_sb[:], ones_dram[:])

        xn_pool = ctx.enter_context(tc.tile_pool(name="xn", bufs=8))

        from concourse import library_config
        nc.gpsimd.load_library(library_config.local_scatter)

        nrep = int(os.environ.get("KREPEAT", "1"))
        for s in [s for _ in range(nrep) for s in range(BPC)]:
            # ---- xT [256 c, 4096 pos] via PE transpose of natural tiles ----
            xt0 = xt_pool.tile([128, HW], F32, tag="xt0")
            xt1 = xt_pool.tile([128, HW], F32, tag="xt1")
            for g in range(8):
                xns = []
                for j in range(4):
                    t = 4 * g + j
                    xn = xn_pool.tile([128, C], F32, tag="xn")
                    nc.sync.dma_start(xn[:], x2[s, t * 128:(t + 1) * 128, :])
                    xns.append(xn)
                for half, xt in ((0, xt0), (1, xt1)):
                    px = ps_misc.tile([128, 512], F32, tag="ps")
                    for j in range(4):
                        nc.tensor.matmul(
                            px[:, j * 128:(j + 1) * 128],
                            xns[j][:, half * 128:(half + 1) * 128],
                            id_sb[:], is_transpose=True)
                    nc.vector.tensor_copy(xt[:, g * 512:(g + 1) * 512], px[:])

            # ---- out1d: padded + row-shift-duplicated down-conv output ----
            o1 = o1_pool.tile([128, 66 * 66], F32, tag="o1")
            # zero the pad border (rows 0,65 and cols 0,65 of the 66x66 grid)
            o1v = o1[:].rearrange("p (r c) -> p r c", r=66)
            nc.vector.memset(o1v[:, 0:1, :], 0.0)
            nc.vector.memset(o1v[:, 65:66, :], 0.0)
            nc.vector.memset(o1v[:, :, 0:1], 0.0)
            nc.vector.memset(o1v[:, :, 65:66], 0.0)

            for n in range(8):          # 8 h-rows per 512-chunk
                pd = ps_misc.tile([C4, 512], F32, tag="ps")
                nc.tensor.matmul(pd[:], wd_sb[:, 0:64],
                                 xt0[:, n * 512:(n + 1) * 512],
                                 start=True, stop=False)
                nc.tensor.matmul(pd[:], wd_sb[:, 64:128],
                                 xt1[:, n * 512:(n + 1) * 512],
                                 start=False, stop=True)
                pdv = pd[:].rearrange("p (r c) -> p r c", r=8)
                # interior write + bias, plus the row-shifted duplicate
                nc.vector.tensor_scalar_add(
                    o1v[0:64, 1 + n * 8:9 + n * 8, 1:65], pdv, bd_sb[:])
                nc.vector.tensor_scalar_add(
                    o1v[64:128, n * 8:8 + n * 8, 1:65], pdv, bd_sb[:])

            # ---- enc conv -> enc_out [36, 4096] ----
            enc_sb = enc_pool.tile([NCH, HW], F32, tag="enc")
            for n in range(8):
                pe = ps_misc.tile([NCH, 512], F32, tag="ps")
                first = True
                for dj in range(3):
                    nc.tensor.matmul(
                        pe[:], weA_sb[:, dj * 36:(dj + 1) * 36],
                        o1v[:, n * 8:n * 8 + 8, dj:dj + 64],
                        start=first, stop=False)
                    first = False
                    nc.tensor.matmul(
                        pe[:], weB_sb[:, dj * 36:(dj + 1) * 36],
                        o1v[0:64, n * 8 + 2:n * 8 + 10, dj:dj + 64],
                        start=False, stop=(dj == 2))
                nc.vector.tensor_scalar_add(
                    enc_sb[:, n * 512:(n + 1) * 512], pe[:], be_sb[:])

            # ---- kt transpose: [4096 pos, 36] chunks + softmax ----
            kts = kt_pool.tile([128, 32 * NCH], F32, tag="kts")
            for c in range(32):
                pt = ps_misc.tile([128, NCH], F32, tag="ps")
                nc.tensor.matmul(pt[:], enc_sb[:, c * 128:(c + 1) * 128],
                                 id_sb[0:NCH, 0:NCH], is_transpose=True)
                nc.vector.tensor_copy(kts[:, c * NCH:(c + 1) * NCH], pt[:])

            ea = kt_pool.tile([128, 32 * NCH], F32, tag="ea")
            nc.scalar.activation(ea[:], kts[:],
                                 mybir.ActivationFunctionType.Exp)
            sums = kt_pool.tile([128, 128], F32, tag="sums")
            nc.vector.reduce_sum(sums[:].rearrange("p (c u) -> p c u", u=NU),
                                 ea[:].rearrange("p (c k u) -> p c u k",
                                                 k=NK, u=NU),
                                 axis=mybir.AxisListType.X)
            rec = kt_pool.tile([128, 128], F32, tag="rec")
            nc.vector.reciprocal(rec[:], sums[:])
            kt16 = kt_pool.tile([128, 32 * NCH], F16, tag="kt16")
            recb = rec[:].rearrange("p (c u) -> p c u", u=NU) \
                         .unsqueeze(2).broadcast_to([128, 32, NK, NU])
            nc.vector.tensor_tensor(
                kt16[:].rearrange("p (c k u) -> p c k u", k=NK, u=NU),
                ea[:].rearrange("p (c k u) -> p c k u", k=NK, u=NU),
                recb, mybir.AluOpType.mult)

            # ---- stage kt to DRAM, reload shifted as data_sh ----
            dsh = ds_pool.tile([128, 33 * NCH], F16, tag="dsh")
            dshv = dsh[:].rearrange("p (c j) -> p c j", j=NCH)
            # chunk-32 tail is only ever masked off by idx==-1; keep it finite
            nc.vector.memset(dshv[:, 32:33, :], 0.0)
            if "dsh" not in ablate:
                nc.sync.dma_start(
                    ktd[s, PAD:PAD + HW, :].rearrange("(c p) j -> p c j", p=128),
                    kt16[:].rearrange("p (c j) -> p c j", j=NCH))
                for k in range(NK):
                    sk = 64 * (k // 3 - 1) + (k % 3 - 1)
                    nc.sync.dma_start(
                        dshv[:, 0:32, 4 * k:4 * k + 4],
                        ktd[s, PAD - sk:PAD - sk + HW, 4 * k:4 * k + 4]
                        .rearrange("(c p) u -> p c u", p=128))
            else:
                nc.vector.memset(dshv[:, 0:32, :], 0.0)

            # ---- v = x @ wo (bias added in reassembly), fp16 [128, 256] ----
            vA = v_pool.tile([128, 32 * C], F16, tag="vA")
            for t in range(32):
                pv = ps_v.tile([128, C], F32, tag="psv")
                nc.tensor.matmul(pv[:], xt0[:, t * 128:(t + 1) * 128],
                                 wo_sb[:, 0:256], start=True, stop=False)
                nc.tensor.matmul(pv[:], xt1[:, t * 128:(t + 1) * 128],
                                 wo_sb[:, 256:512], start=False, stop=True)
                nc.vector.tensor_copy(vA[:, t * C:(t + 1) * C], pv[:])

            # ---- reassembly: 128 groups ----
            if "reasm" in ablate:
                continue
            for h in range(H):
                par, border, tA, tB, pB, tlo = _variant_key(h)
                for gw in range(2):
                    w0 = gw * GW
                    vi = _idx_variant_id(par, w0, border)
                    mkt = mkt_pool.tile([128, 256], F16, tag="mkt")
                    if "scatter" not in ablate:
                        nc.gpsimd.local_scatter(
                            mkt[:], dsh[:, tlo * NCH:(tlo + 2) * NCH],
                            idx_sb[:, vi * 72:(vi + 1) * 72],
                            channels=128, num_elems=256, num_idxs=72)
                    else:
                        nc.vector.memset(mkt[:, 0:1], 0.0)
                    po = ps_o.tile([128, C], F32, tag="pso")
                    nc.tensor.matmul(po[:], mkt[:, 0:128],
                                     vA[:, tA * C:(tA + 1) * C],
                                     start=True, stop=False)
                    if tB is not None:
                        nc.tensor.matmul(
                            po[:], mkt[64 * pB:64 * pB + 64, 128:256],
                            vA[64 * pB:64 * pB + 64, tB * C:(tB + 1) * C],
                            start=False, stop=False)
                    # out-conv bias: border patches contribute bias-free, so
                    # v carries no bias and +bo lands here exactly once
                    nc.tensor.matmul(po[:], ones_sb[:], bo_sb[:],
                                     start=False, stop=True)
                    ob = out_pool.tile([128, C], F32, tag="ob")
                    if (h + gw) % 2 == 0:
                        nc.vector.tensor_copy(ob[:], po[:])
                    else:
                        nc.scalar.copy(ob[:], po[:])
                    eng = nc.sync if (h + gw) % 2 == 0 else nc.scalar
                    for ui in range(2):
                        row = (2 * h + ui) * 2 * W + 2 * w0
                        eng.dma_start(out[s, row:row + 64, :],
                                      ob[64 * ui:64 * ui + 64, :])


# ---------------------------------------------------------------------------
# host entry
# ---------------------------------------------------------------------------

def _pack_weights(down_w, down_b, enc_w, enc_b, out_w, out_b):
    wd = np.zeros((128, 128), np.float32)
    wdT = down_w[:, :, 0, 0].T.astype(np.float32)       # [256 c, 64]
    wd[:, 0:64] = wdT[0:128]
    wd[:, 64:128] = wdT[128:256]
    weA = np.zeros((128, 108), np.float32)
    weB = np.zeros((C4, 108), np.float32)
    for dj in range(3):
        weA[0:64, dj * 36:(dj + 1) * 36] = enc_w[:, :, 0, dj].T
        weA[64:128, dj * 36:(dj + 1) * 36] = enc_w[:, :, 1, dj].T
        weB[:, dj * 36:(dj + 1) * 36] = enc_w[:, :, 2, dj].T
    woT = out_w[:, :, 0, 0].T.astype(np.float32)        # [256 c, 256 cout]
    wo = np.zeros((128, 512), np.float32)
    wo[:, 0:256] = woT[0:128]
    wo[:, 256:512] = woT[128:256]
    return {
        "wd": wd, "bd": down_b.reshape(C4, 1).astype(np.float32),
        "weA": weA, "weB": weB,
        "be": enc_b.reshape(NCH, 1).astype(np.float32),
        "wo": wo, "bo": out_b.reshape(1, C).astype(np.float32),
    }


def kernel(x, down_w, down_b, enc_w, enc_b, out_w, out_b):
    global last_result
    if "nc" not in _cache:
        _cache["nc"] = _build_program()
    nc = _cache["nc"]

    x = np.ascontiguousarray(np.asarray(x, np.float32))
    shared = _pack_weights(np.asarray(down_w), np.asarray(down_b),
                           np.asarray(enc_w), np.asarray(enc_b),
                           np.asarray(out_w), np.asarray(out_b))
    in_maps = []
    for i in range(NCORES):
        m = dict(shared)
        m["x2"] = np.ascontiguousarray(x[BPC * i:BPC * (i + 1)])
        in_maps.append(m)

    res = run_bass_kernel_spmd(nc, in_maps, core_ids=list(range(NCORES)),
                               trace=bool(os.environ.get("KTRACE")))
    last_result = res
    return np.concatenate([r["out"] for r in res.results], axis=0)


# revision 35
# speedup vs baseline: 1.7133x; 1.7133x over previous
"""CARAFE forward as a Bass/Tile kernel for 8 Trainium2 NeuronCores.

Problem (per sample, B=16 total, data-parallel 2 samples/core):
  x [4096, 256] -> down 1x1 conv (256->64) -> enc 3x3 conv (64->36)
  -> pixel_shuffle + softmax over 9 patch weights per upsampled pixel
  -> content-adaptive reassembly of out-conv features -> out [16384, 256]

Key algebraic fusion: the trailing 1x1 conv (out_w) commutes with the
reassembly, so we conv first on the 4096-pixel grid (v = x @ wo; the bias
is added as a rank-1 matmul per output group since border patches
contribute bias-free), then reassemble v directly into the final output:
4x fewer conv FLOPs and no y-transpose.

Reassembly is a locally-connected contraction. Per quad of output groups
(two h-rows x two w-halves) we build the banded weight matrices Mkt
[v-rows, (u,delta)] in one gpsimd local_scatter (per-partition scatter),
then run accumulated fp16 matmuls against natural 128-row chunks of v.
The scatter's per-partition source values (kt of each contributing output
pixel) come from staging softmaxed kt through DRAM transposed, so the
shifted reloads are 4 contiguous 8KB runs per patch offset.
"""
import os
import sys
import numpy as np

sys.path.insert(0, "/opt/trn_rl_repo")

import concourse.bass as bass
import concourse.mybir as mybir
import concourse.tile as tile
from concourse.bass_utils import run_bass_kernel_spmd

F32 = mybir.dt.float32
F16 = mybir.dt.float16
I16 = mybir.dt.int16

U, K, C, H, W = 2, 3, 256, 64, 64
HW = H * W                      # 4096
C4 = C // 4                     # 64
NK, NU, NCH = 9, 4, 36
GW = 32                         # output positions per group
PAD = 65                        # kt DRAM position padding (max |shift|)
NCORES = 8
BPC = 2                         # samples per core

# scatter dst layout (fp16 cols), per w-half block of 384:
#   [0:128)   MM-A odd-h    [128:256) MM-A even-h
#   [256:384) MM-B shared: odd-h at partitions 0:64, even-h at 64:128
DBLK = 384

_cache = {}
last_result = None


# ---------------------------------------------------------------------------
# host-side constant tables
# ---------------------------------------------------------------------------

def _build_idx_tables():
    """Three [128, 72] int16 tables: interior quad, h0, h63.

    Data slot s = ct*36 + k*4 + u over chunks (m, m+1) of data_sh.
    Interior quad (h_odd=2m+1, h_even=2m+2):
      di = 2*ct + p//64 - e  for e in {0 (odd), 1 (even)}; valid di in 0..2
      value lands in region (e, ct) at col w0blk + regbase + 64*ui+2*d+uj.
    """
    tables = {}
    for case in ('int', 'h0', 'h63'):
        idx = np.full((128, 72), -1, np.int16)
        for p in range(128):
            for ct in (0, 1):
                for k in range(NK):
                    di, dj = k // 3, k % 3
                    wr = p % 64
                    w_out = wr - (dj - 1)
                    if not 0 <= w_out < W:
                        continue
                    if case == 'int':
                        e = 2 * ct + p // 64 - di
                        if e not in (0, 1):
                            continue
                        if e == 0:
                            reg = 0 if ct == 0 else 256      # A-odd | B-odd
                            if ct == 1 and p >= 64:
                                continue
                        else:
                            reg = 128 if ct == 1 else 256    # A-even | B-even
                            if ct == 0 and p < 64:
                                continue
                    else:
                        # single h row: only chunk ct=0 contributes (A only)
                        if ct != 0:
                            continue
                        if case == 'h0':
                            dh = p // 64          # rows (0,1) of chunk 0
                        else:
                            dh = p // 64 - 1      # rows (62,63) of chunk 31
                        if di - 1 != dh:
                            continue
                        reg = 0
                    w0 = 0 if w_out < GW else GW
                    d = w_out - w0
                    for u in range(NU):
                        col = (0 if w0 == 0 else DBLK) + reg + \
                            64 * (u // 2) + 2 * d + (u % 2)
                        idx[p, ct * 36 + k * 4 + u] = col
        tables[case] = idx
    return tables


# ---------------------------------------------------------------------------
# device program
# ---------------------------------------------------------------------------

def _build_program():
    nc = bass.Bass()

    x2 = nc.declare_dram_parameter("x2", [BPC, HW, C], F32, isOutput=False)
    wd = nc.declare_dram_parameter("wd", [128, 128], F16, isOutput=False)
    bd = nc.declare_dram_parameter("bd", [C4, 1], F32, isOutput=False)
    weA = nc.declare_dram_parameter("weA", [128, 108], F16, isOutput=False)
    weB = nc.declare_dram_parameter("weB", [C4, 108], F16, isOutput=False)
    be = nc.declare_dram_parameter("be", [NCH, 1], F32, isOutput=False)
    wo = nc.declare_dram_parameter("wo", [128, 512], F16, isOutput=False)
    bo = nc.declare_dram_parameter("bo", [128, C], F32, isOutput=False)
    out = nc.declare_dram_parameter("out", [BPC, 4 * HW, C], F32, True)

    idx_np = _build_idx_tables()
    idx_all = np.concatenate([idx_np[k] for k in ('int', 'h0', 'h63')], axis=1)
    idx_dram = nc.inline_tensor(idx_all, name="idxtab")          # [128, 216]
    ident_dram = nc.inline_tensor(np.eye(128, dtype=np.float32), name="ident")
    ones_dram = nc.inline_tensor(np.ones((1, 128), np.float32), name="ones1")

    with tile.TileContext(nc) as tc:
        _emit(tc, nc, x2, wd, bd, weA, weB, be, wo, bo, out,
              idx_dram, ident_dram, ones_dram)
    # raw-Bass path skips Bacc's extended-inst codegen; without this the
    # NEFF compiler sees empty .instr bytes -> "ISA wrong length"
    from concourse.library_overlay import lower_extended_insts
    lower_extended_insts(nc)
    _split_excess_waits(nc)
    return nc


def _split_excess_waits(nc, cap=1):
    """Each TPB instruction has a single EVENTS wait slot; walrus rejects
    multi-wait instructions ("Too many sync wait commands"). Move excess
    waits onto same-engine NoOps immediately before the instruction —
    semantically identical since the engine blocks at the same PC."""
    nid = [0]
    for f in nc.m.functions:
        for b in f.blocks:
            insts = b.instructions
            i = 0
            while i < len(insts):
                ins = insts[i]
                si = getattr(ins, 'sync_info', None)
                if si is not None and si.on_wait and len(si.on_wait) > cap:
                    waits = list(si.on_wait)
                    for w in waits[:-cap]:
                        nop = mybir.InstNoOp(name=f"nopw-{nid[0]}", ins=[],
                                             outs=[])
                        nid[0] += 1
                        nop.engine = ins.engine
                        nop.sync_info = mybir.SyncInfo(on_wait=[w],
                                                       on_update=[])
                        insts.insert(i, nop)
                        i += 1
                    ins.sync_info = mybir.SyncInfo(
                        on_wait=waits[-cap:],
                        on_update=list(si.on_update or []))
                i += 1


def _emit(tc, nc, x2, wd, bd, weA, weB, be, wo, bo, out,
          idx_dram, ident_dram, ones_dram):
    from contextlib import ExitStack
    ablate = set(os.environ.get("KABLATE", "").split(","))
    ctx = ExitStack()
    with ctx:
        consts = ctx.enter_context(tc.tile_pool(name="consts", bufs=1))
        xt_pool = ctx.enter_context(tc.tile_pool(name="xt", bufs=2))
        o1_pool = ctx.enter_context(tc.tile_pool(name="o1", bufs=1))
        enc_pool = ctx.enter_context(tc.tile_pool(name="enc", bufs=1))
        kt_pool = ctx.enter_context(tc.tile_pool(name="kt", bufs=1))
        ds_pool = ctx.enter_context(tc.tile_pool(name="ds", bufs=2))
        ds9_pool = ctx.enter_context(tc.tile_pool(name="ds9", bufs=1))
        v_pool = ctx.enter_context(tc.tile_pool(name="v", bufs=2))
        mkt_pool = ctx.enter_context(tc.tile_pool(name="mkt", bufs=4))
        out_pool = ctx.enter_context(tc.tile_pool(name="ob", bufs=6))
        xn_pool = ctx.enter_context(tc.tile_pool(name="xn", bufs=8))
        ps_x = ctx.enter_context(tc.tile_pool(name="psx", bufs=2, space="PSUM"))
        ps_dek = ctx.enter_context(tc.tile_pool(name="psdek", bufs=2, space="PSUM"))
        ps_v = ctx.enter_context(tc.tile_pool(name="psv", bufs=2, space="PSUM"))
        ps_o = ctx.enter_context(tc.tile_pool(name="pso", bufs=2, space="PSUM"))

        # ---- constants to SBUF ----
        wd_sb = consts.tile([128, 128], F16)
        nc.sync.dma_start(wd_sb[:], wd[:])
        bd_sb = consts.tile([C4, 1], F32)
        nc.sync.dma_start(bd_sb[:], bd[:])
        weA_sb = consts.tile([128, 108], F16)
        nc.sync.dma_start(weA_sb[:], weA[:])
        weB_sb = consts.tile([C4, 108], F16)
        nc.sync.dma_start(weB_sb[:], weB[:])
        be_sb = consts.tile([NCH, 1], F32)
        nc.sync.dma_start(be_sb[:], be[:])
        wo_sb = consts.tile([128, 512], F16)
        nc.sync.dma_start(wo_sb[:], wo[:])
        bo_sb = consts.tile([128, C], F32)
        nc.sync.dma_start(bo_sb[:], bo[:])
        idx_sb = consts.tile([128, 216], I16)
        nc.sync.dma_start(idx_sb[:], idx_dram[:])
        id_sb = consts.tile([128, 128], F32)
        nc.sync.dma_start(id_sb[:], ident_dram[:])
        ones_sb = consts.tile([1, 128], F32)
        nc.sync.dma_start(ones_sb[:], ones_dram[:])

        from concourse import library_config
        nc.gpsimd.load_library(library_config.local_scatter)

        nrep = int(os.environ.get("KREPEAT", "1"))
        for s in [s for _ in range(nrep) for s in range(BPC)]:
            # ---- xT [256 c, 4096 pos] via PE transpose of natural tiles ----
            xt0 = xt_pool.tile([128, HW], F16, tag="xt0")
            xt1 = xt_pool.tile([128, HW], F16, tag="xt1")
            for g in range(8):
                xns = []
                for j in range(4):
                    t = 4 * g + j
                    xn = xn_pool.tile([128, C], F32, tag="xn")
                    nc.sync.dma_start(xn[:], x2[s, t * 128:(t + 1) * 128, :])
                    xns.append(xn)
                for half, xt in ((0, xt0), (1, xt1)):
                    px = ps_x.tile([128, 512], F32, tag="psx")
                    for j in range(4):
                        nc.tensor.matmul(
                            px[:, j * 128:(j + 1) * 128],
                            xns[j][:, half * 128:(half + 1) * 128],
                            id_sb[:], is_transpose=True)
                    nc.vector.tensor_copy(xt[:, g * 512:(g + 1) * 512], px[:])

            # ---- out1d: padded + row-shift-duplicated down-conv output ----
            o1 = o1_pool.tile([128, 66 * 66], F16, tag="o1")
            o1v = o1[:].rearrange("p (r c) -> p r c", r=66)
            nc.vector.memset(o1v[:, 0:1, :], 0.0)
            nc.vector.memset(o1v[:, 65:66, :], 0.0)
            nc.vector.memset(o1v[:, :, 0:1], 0.0)
            nc.vector.memset(o1v[:, :, 65:66], 0.0)

            for n in range(8):          # 8 h-rows per 512-chunk
                pd = ps_dek.tile([C4, 512], F32, tag="psdek")
                nc.tensor.matmul(pd[:], wd_sb[:, 0:64],
                                 xt0[:, n * 512:(n + 1) * 512],
                                 start=True, stop=False)
                nc.tensor.matmul(pd[:], wd_sb[:, 64:128],
                                 xt1[:, n * 512:(n + 1) * 512],
                                 start=False, stop=True)
                pdv = pd[:].rearrange("p (r c) -> p r c", r=8)
                nc.vector.tensor_scalar_add(
                    o1v[0:64, 1 + n * 8:9 + n * 8, 1:65], pdv, bd_sb[:])
                nc.vector.tensor_scalar_add(
                    o1v[64:128, n * 8:8 + n * 8, 1:65], pdv, bd_sb[:])

            # ---- enc conv -> enc_out [36, 4096] ----
            enc_sb = enc_pool.tile([NCH, HW], F32, tag="enc")
            for n in range(8):
                pe = ps_dek.tile([NCH, 512], F32, tag="psdek")
                first = True
                for dj in range(3):
                    nc.tensor.matmul(
                        pe[:], weA_sb[:, dj * 36:(dj + 1) * 36],
                        o1v[:, n * 8:n * 8 + 8, dj:dj + 64],
                        start=first, stop=False)
                    first = False
                    nc.tensor.matmul(
                        pe[:], weB_sb[:, dj * 36:(dj + 1) * 36],
                        o1v[0:64, n * 8 + 2:n * 8 + 10, dj:dj + 64],
                        start=False, stop=(dj == 2))
                nc.vector.tensor_scalar_add(
                    enc_sb[:, n * 512:(n + 1) * 512], pe[:], be_sb[:])

            # ---- kt transpose: [4096 pos, 36] chunks + softmax ----
            kts = kt_pool.tile([128, 32 * NCH], F32, tag="kts")
            for c in range(32):
                pt = ps_dek.tile([128, NCH], F32, tag="psdek")
                nc.tensor.matmul(pt[:], enc_sb[:, c * 128:(c + 1) * 128],
                                 id_sb[0:NCH, 0:NCH], is_transpose=True)
                nc.vector.tensor_copy(kts[:, c * NCH:(c + 1) * NCH], pt[:])

            ea = kt_pool.tile([128, 32 * NCH], F32, tag="ea")
            nc.scalar.activation(ea[:], kts[:],
                                 mybir.ActivationFunctionType.Exp)
            sums = kt_pool.tile([128, 128], F32, tag="sums")
            nc.vector.reduce_sum(sums[:].rearrange("p (c u) -> p c u", u=NU),
                                 ea[:].rearrange("p (c k u) -> p c u k",
                                                 k=NK, u=NU),
                                 axis=mybir.AxisListType.X)
            rec = kt_pool.tile([128, 128], F32, tag="rec")
            nc.vector.reciprocal(rec[:], sums[:])
            # kt16p: 35 chunk-blocks of 36 fp16; blocks 1..32 hold the 32
            # position-chunks, blocks 0/33/34 are zero halo for the shifts
            kt16p = kt_pool.tile([128, 35 * NCH], F16, tag="kt16")
            nc.vector.memset(kt16p[:, 0:NCH], 0.0)
            nc.vector.memset(kt16p[:, 33 * NCH:35 * NCH], 0.0)
            recb = rec[:].rearrange("p (c u) -> p c u", u=NU) \
                         .unsqueeze(2).broadcast_to([128, 32, NK, NU])
            nc.vector.tensor_tensor(
                kt16p[:, NCH:33 * NCH].rearrange("p (c k u) -> p c k u",
                                                 k=NK, u=NU),
                ea[:].rearrange("p (c k u) -> p c k u", k=NK, u=NU),
                recb, mybir.AluOpType.mult)

            # ---- data_sh via partition-shifted SBUF->SBUF copies ----
            # dsh9[p, (sigma, c, j)] = kt16p shifted by s_k on the flat
            # position index; then one strided DVE repack compacts the
            # sigma==k slots to dsh[p, (c, k, u)].
            dsh9 = ds9_pool.tile([128, NK * 33 * NCH], F16, tag="dsh9")
            if "dsh" not in ablate:
                for k in range(NK):
                    sk = 64 * (k // 3 - 1) + (k % 3 - 1)
                    blk = k * 33 * NCH
                    span = 33 * NCH
                    if sk == 0:
                        nc.scalar.dma_start(
                            dsh9[:, blk:blk + span],
                            kt16p[:, NCH:NCH + span])
                        continue
                    if sk > 0:
                        # dst p in [sk,128) <- src (p-sk, chunk c) ;
                        # dst p in [0,sk)   <- src (p-sk+128, chunk c-1)
                        nc.scalar.dma_start(
                            dsh9[sk:128, blk:blk + span],
                            kt16p[0:128 - sk, NCH:NCH + span])
                        nc.scalar.dma_start(
                            dsh9[0:sk, blk:blk + span],
                            kt16p[128 - sk:128, 0:span])
                    else:
                        nc.scalar.dma_start(
                            dsh9[0:128 + sk, blk:blk + span],
                            kt16p[-sk:128, NCH:NCH + span])
                        nc.scalar.dma_start(
                            dsh9[128 + sk:128, blk:blk + span],
                            kt16p[0:-sk, 2 * NCH:2 * NCH + span])
            else:
                nc.vector.memset(dsh9[:, 0:1], 0.0)
            dsh = ds_pool.tile([128, 33 * NCH], F16, tag="dsh")
            # repack: dsh[p, c*36 + k*4 + u] = dsh9[p, k-block, c, 4k+u]
            dshv = dsh[:].rearrange("p (c j) -> p c j", j=NCH)
            d9v = dsh9[:].rearrange("p (k c j) -> p k c j", k=NK, j=NCH)
            for k in range(NK):
                nc.vector.tensor_copy(dshv[:, :, 4 * k:4 * k + 4],
                                      d9v[:, k, :, 4 * k:4 * k + 4])

            # ---- v = x @ wo (bias in reassembly), fp16 [128, 256]x32 ----
            vA = v_pool.tile([128, 32 * C], F16, tag="vA")
            for t in range(32):
                pv = ps_v.tile([128, C], F32, tag="psv")
                nc.tensor.matmul(pv[:], xt0[:, t * 128:(t + 1) * 128],
                                 wo_sb[:, 0:256], start=True, stop=False)
                nc.tensor.matmul(pv[:], xt1[:, t * 128:(t + 1) * 128],
                                 wo_sb[:, 256:512], start=False, stop=True)
                nc.vector.tensor_copy(vA[:, t * C:(t + 1) * C], pv[:])

            # ---- reassembly: 31 interior quads + h0 + h63 ----
            if "reasm" in ablate:
                continue

            def emit_group(h, mkt):
                """matmuls for both w-halves of row h into one wide PSUM
                bank, fused bias add, one output DMA per subrow ui."""
                par = h % 2
                if par == 1:
                    tA, tB, pB = (h - 1) // 2, (h + 1) // 2, 0
                else:
                    tA, tB, pB = h // 2, h // 2 - 1, 1
                if h == 0:
                    tA, tB = 0, None
                if h == H - 1:
                    tB = None
                po = ps_o.tile([128, 2 * C], F32, tag="pso")
                for wi, blk in ((0, 0), (1, DBLK)):
                    if h in (0, H - 1):
                        areg = blk
                    else:
                        areg = blk + (0 if par == 1 else 128)
                    nc.tensor.matmul(po[:, wi * C:(wi + 1) * C],
                                     mkt[:, areg:areg + 128],
                                     vA[:, tA * C:(tA + 1) * C],
                                     start=True, stop=(tB is None))
                    if tB is not None:
                        nc.tensor.matmul(
                            po[64 * pB:64 * pB + 64, wi * C:(wi + 1) * C]
                            if False else po[:, wi * C:(wi + 1) * C],
                            mkt[64 * pB:64 * pB + 64,
                                blk + 256:blk + 384],
                            vA[64 * pB:64 * pB + 64, tB * C:(tB + 1) * C],
                            start=False, stop=True)
                # out-conv bias: border patches contribute bias-free, so v
                # carries no bias and +bo (materialized broadcast tile) lands
                # here exactly once, fused into the PSUM->SBUF copy
                ob = out_pool.tile([128, 2 * C], F32, tag="ob")
                bo2 = bo_sb[:].unsqueeze(1).broadcast_to([128, 2, C])
                nc.vector.tensor_tensor(
                    ob[:].rearrange("p (w c) -> p w c", w=2), po[:]
                    .rearrange("p (w c) -> p w c", w=2),
                    bo2, mybir.AluOpType.add)
                eng = nc.sync if h % 2 == 0 else nc.scalar
                for ui in range(2):
                    base = (2 * h + ui) * 2 * W
                    dst = out[s, base:base + 128, :] \
                        .rearrange("(w q) c -> q w c", w=2)
                    eng.dma_start(dst,
                                  ob[64 * ui:64 * ui + 64, :]
                                  .rearrange("p (w c) -> p w c", w=2))

            def emit_scatter(tlo, variant, ncols):
                mkt = mkt_pool.tile([128, 2 * DBLK], F16, tag="mkt")
                if "scatter" not in ablate:
                    nc.gpsimd.local_scatter(
                        mkt[:, 0:ncols], dsh[:, tlo * NCH:(tlo + 2) * NCH],
                        idx_sb[:, variant * 72:(variant + 1) * 72],
                        channels=128, num_elems=ncols, num_idxs=72)
                else:
                    nc.vector.memset(mkt[:, 0:1], 0.0)
                return mkt

            # h = 0 (A-only; dst uses regions [0:128] and [DBLK:DBLK+128])
            mkt = emit_scatter(0, 1, 2 * DBLK)
            emit_group(0, mkt)
            # interior quads m: h = 2m+1, 2m+2
            for m in range(31):
                mkt = emit_scatter(m, 0, 2 * DBLK)
                emit_group(2 * m + 1, mkt)
                emit_group(2 * m + 2, mkt)
            # h = 63
            mkt = emit_scatter(31, 2, 2 * DBLK)
            emit_group(63, mkt)


# ---------------------------------------------------------------------------
# host entry
# ---------------------------------------------------------------------------

def _pack_weights(down_w, down_b, enc_w, enc_b, out_w, out_b):
    wd = np.zeros((128, 128), np.float32)
    wdT = down_w[:, :, 0, 0].T.astype(np.float32)       # [256 c, 64]
    wd[:, 0:64] = wdT[0:128]
    wd[:, 64:128] = wdT[128:256]
    weA = np.zeros((128, 108), np.float32)
    weB = np.zeros((C4, 108), np.float32)
    for dj in range(3):
        weA[0:64, dj * 36:(dj + 1) * 36] = enc_w[:, :, 0, dj].T
        weA[64:128, dj * 36:(dj + 1) * 36] = enc_w[:, :, 1, dj].T
        weB[:, dj * 36:(dj + 1) * 36] = enc_w[:, :, 2, dj].T
    woT = out_w[:, :, 0, 0].T.astype(np.float32)        # [256 c, 256 cout]
    wo = np.zeros((128, 512), np.float32)
    wo[:, 0:256] = woT[0:128]
    wo[:, 256:512] = woT[128:256]
    bo_bc = np.broadcast_to(out_b.reshape(1, C), (128, C)).astype(np.float32)
    return {
        "wd": wd.astype(np.float16), "bd": down_b.reshape(C4, 1).astype(np.float32),
        "weA": weA.astype(np.float16), "weB": weB.astype(np.float16),
        "be": enc_b.reshape(NCH, 1).astype(np.float32),
        "wo": wo.astype(np.float16),
        "bo": np.ascontiguousarray(bo_bc),
    }


def kernel(x, down_w, down_b, enc_w, enc_b, out_w, out_b):
    global last_result
    if "nc" not in _cache:
        _cache["nc"] = _build_program()
    nc = _cache["nc"]

    x = np.ascontiguousarray(np.asarray(x, np.float32))
    shared = _pack_weights(np.asarray(down_w), np.asarray(down_b),
                           np.asarray(enc_w), np.asarray(enc_b),
                           np.asarray(out_w), np.asarray(out_b))
    in_maps = []
    for i in range(NCORES):
        m = dict(shared)
        m["x2"] = np.ascontiguousarray(x[BPC * i:BPC * (i + 1)])
        in_maps.append(m)

    res = run_bass_kernel_spmd(nc, in_maps, core_ids=list(range(NCORES)),
                               trace=bool(os.environ.get("KTRACE")))
    last_result = res
    return np.concatenate([r["out"] for r in res.results], axis=0)
